# revision 13
# baseline (speedup 1.0000x reference)
"""LAGCNII on 8 TRN2 NeuronCores — full on-device implementation.

Architecture ("AG-pull"):
  - Nodes sharded 12500/core. Each core's nodes are bin-packed into 100
    windows of <=128 "slots" (3 gather-chunks of 128 edge-rows per window).
  - Per layer: AllGather replicates every core's h slab (bf16, slot-order)
    into a 102400-row arena; each core pull-gathers its edges' source rows
    in consumer order (one 128-row indirect DMA per chunk — the SWDGE
    per-partition-offset primitive), then a one-hot R matmul (bf16,
    SBUF-resident, generated once on DVE) computes the segment sum into
    PSUM. DVE adds the folded initial-residual (h0/9), PE transposes x,
    and the dense matmul uses W''_l = 0.9*((1-beta_l)I + beta_l*W_l) so
    the whole GCN2Conv layer is agg -> (+h0/9) -> @W'' -> relu.
  - Layer 8 fuses the output projection (h8 @ out_w); out_b added on host.

Math identity used:  x = 0.9*agg + 0.1*h0 = 0.9*(agg + h0/9)
                     out = x@((1-b)I + bW) = (agg + h0/9) @ W''  (exact)
"""
import numpy as np
import ml_dtypes

N = 100000
E = 300000
D_IN = 256
CH = 256
L = 8
C = 40
ALPHA = 0.1
THETA = 0.5

N_CORES = 8
NPC = N // N_CORES          # 12500 nodes per core
NW = 100                    # windows per core
SLOTS = NW * 128            # 12800 node-slots per core
CPW = 3                     # gather chunks per window (late windows)
D_EARLY = 16                # windows with 4 quarter-pure chunks
CHUNKS = D_EARLY * 4 + (NW - D_EARLY) * CPW   # 316
ROWCAP = CPW * 128          # 384 edge-rows per window
CHUNK_BASE = [w * 4 if w < D_EARLY else
              D_EARLY * 4 + (w - D_EARLY) * CPW for w in range(NW)]
N_CHUNKS_W = [4 if w < D_EARLY else CPW for w in range(NW)]
ARENA = N_CORES * SLOTS     # 102400
AGQ = 4                     # AllGather split count
QR = SLOTS // AGQ           # 3200 rows per AG split

_cached = {}


# ----------------------------------------------------------------------
# host-side graph prep
# ----------------------------------------------------------------------

def _prep_graph(edge_index):
    src = np.asarray(edge_index[0], dtype=np.int64)
    dst = np.asarray(edge_index[1], dtype=np.int64)
    owner = dst // NPC

    slot_of = np.empty(N, dtype=np.int64)      # global node -> slot in owner slab
    win_of_node = np.empty(N, dtype=np.int64)  # global node -> window (per owner)
    msl_of_node = np.empty(N, dtype=np.int64)  # global node -> slot-in-window
    per_core = []

    indeg_all = np.bincount(dst, minlength=N)
    for c in range(N_CORES):
        lo = c * NPC
        deg = indeg_all[lo:lo + NPC]
        order = np.argsort(-deg, kind="stable")
        win = np.empty(NPC, dtype=np.int64)
        msl = np.empty(NPC, dtype=np.int64)
        # LPT greedy: node with largest in-degree -> least-loaded window
        import heapq
        heap = [(0, wi) for wi in range(NW)]
        wcount = [0] * NW
        deg_l = deg.tolist()
        for n in order.tolist():
            while True:
                load, wi = heapq.heappop(heap)
                if wcount[wi] < 128:
                    break
            win[n] = wi
            msl[n] = wcount[wi]
            wcount[wi] += 1
            heapq.heappush(heap, (load + deg_l[n], wi))
        wsum = np.bincount(win, weights=deg.astype(np.float64), minlength=NW)
        assert wsum.max() <= ROWCAP, f"core {c}: window overflow {wsum.max()}"
        slot_of[lo:lo + NPC] = win * 128 + msl
        win_of_node[lo:lo + NPC] = win
        msl_of_node[lo:lo + NPC] = msl
        per_core.append([win, msl])

    # relabel windows per core so the first D_EARLY labels are windows whose
    # sources are quarter-balanced (<=120 per arena quarter), enabling
    # quarter-pure early chunks
    qday = N_CORES * QR
    for c in range(N_CORES):
        m = owner == c
        s_e = src[m]
        w_e = win_of_node[dst[m]]
        sl = slot_of[s_e]
        arow = (sl // QR) * qday + (s_e // NPC) * QR + (sl % QR)
        qtr = arow // qday
        qc = np.zeros((NW, AGQ), dtype=np.int64)
        np.add.at(qc, (w_e, qtr), 1)
        ok = np.where((qc <= 120).all(axis=1))[0]
        assert len(ok) >= D_EARLY, f"core {c}: only {len(ok)} balanced windows"
        rest = np.setdiff1d(np.arange(NW), ok[:D_EARLY])
        perm = np.empty(NW, dtype=np.int64)
        perm[ok[:D_EARLY]] = np.arange(D_EARLY)
        perm[rest] = np.arange(D_EARLY, NW)
        win, msl = per_core[c]
        win = perm[win]
        per_core[c] = [win, msl]
        lo = c * NPC
        slot_of[lo:lo + NPC] = win * 128 + msl
        win_of_node[lo:lo + NPC] = win

    gidx = np.zeros((N_CORES, 128, CHUNKS), dtype=np.int32)
    dstcol = np.full((N_CORES, 128, CHUNKS), -1.0, dtype=np.float32)
    for c in range(N_CORES):
        m = owner == c
        s_e = src[m]
        d_e = dst[m]
        w_e = win_of_node[d_e]
        o_e = np.argsort(w_e, kind="stable")
        s_e, d_e, w_e = s_e[o_e], d_e[o_e], w_e[o_e]
        # arena layout: [quarter][rank][quarter-slab] so each AllGather
        # split writes a contiguous region
        sl = slot_of[s_e]
        arow = (sl // QR) * (N_CORES * QR) + (s_e // NPC) * QR + (sl % QR)
        # within each window, order edges by arena row so early chunks
        # reference only early AllGather quarters (tighter dep bounds)
        o2 = np.lexsort((arow, w_e))
        s_e, d_e, w_e, arow = s_e[o2], d_e[o2], w_e[o2], arow[o2]
        qday = N_CORES * QR
        qtr = arow // qday
        cnt = np.bincount(w_e, minlength=NW)
        starts = np.concatenate(([0], np.cumsum(cnt)[:-1]))
        pos = np.arange(len(w_e)) - starts[w_e]
        assert pos.max() < ROWCAP
        early = w_e < D_EARLY
        # early windows: chunk = quarter (quarter-pure); position within
        # chunk = running index within (window, quarter)
        qstart = np.zeros((NW, AGQ), dtype=np.int64)
        qcnt = np.zeros((NW, AGQ), dtype=np.int64)
        np.add.at(qcnt, (w_e, qtr), 1)
        qpos = np.empty(len(w_e), dtype=np.int64)
        # edges are sorted by (w, arow) => within a window, by quarter
        qstart[:, 1:] = np.cumsum(qcnt, axis=1)[:, :-1]
        qpos = pos - qstart[w_e, qtr]
        assert qpos[early].max(initial=0) < 128
        chunk = np.where(
            early,
            np.asarray(CHUNK_BASE)[w_e] + qtr,
            np.asarray(CHUNK_BASE)[w_e] + pos // 128)
        part = np.where(early, qpos, pos % 128)
        gidx[c, part, chunk] = arow.astype(np.int32)
        dstcol[c, part, chunk] = msl_of_node[d_e].astype(np.float32)
    return slot_of, per_core, gidx, dstcol


# ----------------------------------------------------------------------
# device kernel builder
# ----------------------------------------------------------------------

def _build_kernel(chunk_bound):
    if "nc" in _cached:
        return _cached["nc"]
    import concourse.bacc as bacc
    import concourse.mybir as mybir
    import concourse.tile as tile
    import concourse.bass as bass

    AF = mybir.ActivationFunctionType
    nc = bacc.Bacc("TRN2", target_bir_lowering=False, debug=False,
                   num_devices=N_CORES)

    x0T = nc.dram_tensor("x0T", [CH, SLOTS], mybir.dt.bfloat16, kind="ExternalInput")
    x1T = nc.dram_tensor("x1T", [CH, SLOTS], mybir.dt.bfloat16, kind="ExternalInput")
    gidx = nc.dram_tensor("gidx", [128, CHUNKS], mybir.dt.int32, kind="ExternalInput")
    dcol = nc.dram_tensor("dcol", [128, CHUNKS], mybir.dt.float32,
                          kind="ExternalInput")
    iota = nc.dram_tensor("iota", [128, 128], mybir.dt.float32, kind="ExternalInput")
    idf = nc.dram_tensor("idf", [128, 128], mybir.dt.float32, kind="ExternalInput")
    idb = nc.dram_tensor("idb", [128, 128], mybir.dt.bfloat16, kind="ExternalInput")
    linw = nc.dram_tensor("linw", [CH, 256], mybir.dt.bfloat16, kind="ExternalInput")
    wpp = nc.dram_tensor("wpp", [L * CH, CH], mybir.dt.float32, kind="ExternalInput")
    outw = nc.dram_tensor("outw", [CH, C], mybir.dt.bfloat16, kind="ExternalInput")
    o = nc.dram_tensor("o", [SLOTS, C], mybir.dt.float32, kind="ExternalOutput")

    agin = [[nc.dram_tensor(f"agin{i}q{q}", [QR, CH], mybir.dt.bfloat16)
             for q in range(AGQ)] for i in range(2)]
    agout = [nc.dram_tensor(f"agout{i}", [ARENA, CH], mybir.dt.bfloat16,
                            addr_space="Shared")
             for i in range(2)]

    RG = [list(range(N_CORES))]

    def emit_ag_q(src_q, dst_t, q):
        # arena layout [quarter][rank][quarter-slab]: split q's output is
        # the contiguous row range [q*8*QR, (q+1)*8*QR)
        nc.gpsimd.collective_compute(
            "AllGather", mybir.AluOpType.bypass, replica_groups=RG,
            ins=[src_q[:].opt()],
            outs=[dst_t[q * N_CORES * QR:(q + 1) * N_CORES * QR, :].opt()],
        )

    with tile.TileContext(nc) as tc:
        with (
            tc.tile_pool(name="cst", bufs=1) as cst,
            tc.tile_pool(name="gp", bufs=20) as gp,
            tc.tile_pool(name="xp", bufs=4) as xp,
            tc.tile_pool(name="lp", bufs=4) as lp,
            tc.tile_pool(name="ps", bufs=2, space="PSUM") as ps,
            tc.tile_pool(name="psa", bufs=4, space="PSUM") as psa,
        ):
            # ---- constants ----
            idx_sb = cst.tile([128, CHUNKS], mybir.dt.int32, tag="idx")
            nc.sync.dma_start(out=idx_sb[:], in_=gidx[:])
            dc_sb = cst.tile([128, CHUNKS], mybir.dt.float32, tag="dc")
            nc.sync.dma_start(out=dc_sb[:], in_=dcol[:])
            iota_sb = cst.tile([128, 128], mybir.dt.float32, tag="iota")
            nc.sync.dma_start(out=iota_sb[:], in_=iota[:])
            idf_sb = cst.tile([128, 128], mybir.dt.float32, tag="idf")
            nc.sync.dma_start(out=idf_sb[:], in_=idf[:])
            idb_sb = cst.tile([128, 128], mybir.dt.bfloat16, tag="idb")
            nc.sync.dma_start(out=idb_sb[:], in_=idb[:])
            lw_sb = cst.tile([128, 512], mybir.dt.bfloat16, tag="lw")
            nc.sync.dma_start(out=lw_sb[:, 0:256], in_=linw[0:128, :])
            nc.sync.dma_start(out=lw_sb[:, 256:512], in_=linw[128:256, :])
            w_sb = cst.tile([128, L * 2 * 256], mybir.dt.float32, tag="wpp")
            for l in range(L):
                for ci in range(2):
                    nc.sync.dma_start(
                        out=w_sb[:, (2 * l + ci) * 256:(2 * l + ci + 1) * 256],
                        in_=wpp[l * CH + ci * 128:l * CH + (ci + 1) * 128, :])
            ow_sb = cst.tile([128, 2 * C], mybir.dt.bfloat16, tag="ow")
            nc.sync.dma_start(out=ow_sb[:, 0:C], in_=outw[0:128, :])
            nc.sync.dma_start(out=ow_sb[:, C:2 * C], in_=outw[128:256, :])

            rbig = cst.tile([128, CHUNKS * 128], mybir.dt.bfloat16, tag="rbig")
            h0c = cst.tile([128, NW * 256], mybir.dt.bfloat16, tag="h0c")
            part = cst.tile([128, D_EARLY * 256], mybir.dt.float32, tag="part")

            # ---- R generation (one is_equal per chunk; layer-invariant) ----
            for k in range(CHUNKS):
                nc.vector.tensor_tensor(
                    out=rbig[:, k * 128:(k + 1) * 128],
                    in0=dc_sb[:, k:k + 1].to_broadcast([128, 128]),
                    in1=iota_sb[:],
                    op=mybir.AluOpType.is_equal)

            # ---- lin stage: h0 = relu([x0@W0, x1@W1]); also h0c = h0/9 ----
            for w in range(NW):
                cs = slice(w * 128, (w + 1) * 128)
                h0p = ps.tile([128, 256], mybir.dt.float32, tag="outp")
                for vi, xT in enumerate((x0T, x1T)):
                    ns = slice(vi * 128, (vi + 1) * 128)
                    xt_in = lp.tile([128, 256], mybir.dt.bfloat16, tag="xin")
                    nc.sync.dma_start(
                        out=xt_in[:],
                        in_=xT[:].rearrange("(a p) s -> p a s", p=128)[:, :, cs])
                    for ci in range(2):
                        nc.tensor.matmul(
                            out=h0p[:, ns], lhsT=xt_in[:, ci * 128:(ci + 1) * 128],
                            rhs=lw_sb[:, (ci * 256 + vi * 128):(ci * 256 + vi * 128 + 128)],
                            start=(ci == 0), stop=(ci == 1))
                h0t = xp.tile([128, 256], mybir.dt.bfloat16, tag="hnew")
                nc.scalar.activation(out=h0t[:], in_=h0p[:], func=AF.Relu)
                q0, r0 = divmod(w, NW // AGQ)
                nc.sync.dma_start(
                    out=agin[0][q0][r0 * 128:(r0 + 1) * 128, :], in_=h0t[:])
                nc.scalar.activation(out=h0c[:, w * 256:(w + 1) * 256],
                                     in_=h0p[:], func=AF.Relu, scale=1.0 / 9.0)
                if r0 == NW // AGQ - 1:
                    emit_ag_q(agin[0][q0], agout[0], q0)

            # ---- per-window tail: x -> transpose -> dense -> relu -> out ----
            def finish_window(l, w, x):
                xtp = ps.tile([128, 256], mybir.dt.float32, tag="xtp")
                nc.tensor.transpose(out=xtp[:, 0:128], in_=x[:, 0:128],
                                    identity=idf_sb[:])
                nc.tensor.transpose(out=xtp[:, 128:256], in_=x[:, 128:256],
                                    identity=idf_sb[:])
                xt = xp.tile([128, 256], mybir.dt.float32, tag="xt")
                nc.vector.tensor_copy(out=xt[:], in_=xtp[:])
                outp = ps.tile([128, 256], mybir.dt.float32, tag="outp")
                nc.tensor.matmul(
                    out=outp[:], lhsT=xt[:, 0:128],
                    rhs=w_sb[:, (2 * l) * 256:(2 * l + 1) * 256],
                    start=True, stop=False)
                nc.tensor.matmul(
                    out=outp[:], lhsT=xt[:, 128:256],
                    rhs=w_sb[:, (2 * l + 1) * 256:(2 * l + 2) * 256],
                    start=False, stop=True)
                hnew = xp.tile([128, 256], mybir.dt.bfloat16, tag="hnew")
                nc.scalar.activation(out=hnew[:], in_=outp[:], func=AF.Relu)
                if l < L - 1:
                    q0, r0 = divmod(w, NW // AGQ)
                    nc.sync.dma_start(
                        out=agin[(l + 1) % 2][q0][r0 * 128:(r0 + 1) * 128, :],
                        in_=hnew[:])
                    if r0 == NW // AGQ - 1:
                        emit_ag_q(agin[(l + 1) % 2][q0],
                                  agout[(l + 1) % 2], q0)
                else:
                    # fused output projection
                    h8tp = ps.tile([128, 256], mybir.dt.bfloat16, tag="xtp")
                    nc.tensor.transpose(out=h8tp[:, 0:128],
                                        in_=hnew[:, 0:128],
                                        identity=idb_sb[:])
                    nc.tensor.transpose(out=h8tp[:, 128:256],
                                        in_=hnew[:, 128:256],
                                        identity=idb_sb[:])
                    h8t = xp.tile([128, 256], mybir.dt.bfloat16, tag="xt8")
                    nc.vector.tensor_copy(out=h8t[:], in_=h8tp[:])
                    projp = psa.tile([128, C], mybir.dt.float32, tag="agg")
                    nc.tensor.matmul(out=projp[:], lhsT=h8t[:, 0:128],
                                     rhs=ow_sb[:, 0:C],
                                     start=True, stop=False)
                    nc.tensor.matmul(out=projp[:], lhsT=h8t[:, 128:256],
                                     rhs=ow_sb[:, C:2 * C],
                                     start=False, stop=True)
                    oc = xp.tile([128, C], mybir.dt.float32, tag="oc")
                    nc.vector.tensor_copy(out=oc[:], in_=projp[:])
                    nc.sync.dma_start(
                        out=o[w * 128:(w + 1) * 128, :], in_=oc[:])

            # ---- GCN2 layers ----
            for l in range(L):
                src_t = agout[l % 2]

                def gather_mm(ch, aggp, start, stop):
                    g = gp.tile([128, 256], mybir.dt.bfloat16, tag="g")
                    nc.gpsimd.indirect_dma_start(
                        out=g[:], out_offset=None,
                        in_=src_t[0:chunk_bound[ch], :],
                        in_offset=bass.IndirectOffsetOnAxis(
                            ap=idx_sb[:, ch:ch + 1], axis=0))
                    nc.tensor.matmul(
                        out=aggp[:], lhsT=rbig[:, ch * 128:(ch + 1) * 128],
                        rhs=g[:], start=start, stop=stop)

                # early windows: quarter waves so the queue never blocks on a
                # not-yet-arrived AllGather quarter; partials accumulate in SBUF
                for k3 in range(4):
                    for we in range(D_EARLY):
                        ch = CHUNK_BASE[we] + k3
                        aggp = psa.tile([128, 256], mybir.dt.float32, tag="agg")
                        gather_mm(ch, aggp, True, True)
                        ps_ = part[:, we * 256:(we + 1) * 256]
                        if k3 == 0:
                            nc.vector.tensor_copy(out=ps_, in_=aggp[:])
                        elif k3 < 3:
                            nc.vector.tensor_add(out=ps_, in0=ps_, in1=aggp[:])
                        else:
                            x = xp.tile([128, 256], mybir.dt.float32, tag="x")
                            nc.vector.tensor_add(out=x[:], in0=aggp[:], in1=ps_)
                            nc.vector.tensor_add(
                                out=x[:], in0=x[:],
                                in1=h0c[:, we * 256:(we + 1) * 256])
                            finish_window(l, we, x)

                for w in range(D_EARLY, NW):
                    aggp = psa.tile([128, 256], mybir.dt.float32, tag="agg")
                    ncw = N_CHUNKS_W[w]
                    for k3 in range(ncw):
                        gather_mm(CHUNK_BASE[w] + k3, aggp, k3 == 0,
                                  k3 == ncw - 1)
                    x = xp.tile([128, 256], mybir.dt.float32, tag="x")
                    nc.vector.tensor_add(
                        out=x[:], in0=aggp[:],
                        in1=h0c[:, w * 256:(w + 1) * 256])
                    finish_window(l, w, x)


    nc.compile()
    _cached["nc"] = nc
    return nc


# ----------------------------------------------------------------------
# entry point
# ----------------------------------------------------------------------

def kernel(x0, x1, edge_index, lin_w, lin_b, gcn_w, out_w, out_b):
    from concourse import bass_utils

    x0 = np.asarray(x0, dtype=np.float32)
    x1 = np.asarray(x1, dtype=np.float32)
    lin_w = np.asarray(lin_w, dtype=np.float32)
    lin_b = np.asarray(lin_b, dtype=np.float32)
    gcn_w = np.asarray(gcn_w, dtype=np.float32)
    out_w = np.asarray(out_w, dtype=np.float32)
    out_b = np.asarray(out_b, dtype=np.float32)
    assert not np.any(lin_b), "kernel assumes lin_b == 0"

    slot_of, per_core, gidx, dstcol = _prep_graph(edge_index)
    # per-chunk arena row bound (max over cores, SPMD-uniform), rounded to
    # the next AllGather-quarter boundary
    QB = N_CORES * QR
    cb = gidx.max(axis=(0, 1)).astype(np.int64)  # max row per chunk
    chunk_bound = (((cb // QB) + 1) * QB).astype(int).tolist()

    # weights: W''_l = 0.9*((1-b)I + b*W_l)
    betas = np.log(THETA / np.arange(1, L + 1, dtype=np.float64) + 1.0)
    wpp = np.empty((L * CH, CH), dtype=np.float32)
    eye = np.eye(CH, dtype=np.float64)
    for l in range(L):
        wl = 0.9 * ((1.0 - betas[l]) * eye + betas[l] * gcn_w[l].astype(np.float64))
        wpp[l * CH:(l + 1) * CH] = wl.astype(np.float32)

    linw = np.concatenate([lin_w[0], lin_w[1]], axis=1)  # [256, 256]
    iota = np.tile(np.arange(128, dtype=np.float32), (128, 1))
    idf = np.eye(128, dtype=np.float32)

    shared = {
        "iota": iota,
        "idf": idf,
        "idb": idf.astype(ml_dtypes.bfloat16),
        "linw": linw.astype(ml_dtypes.bfloat16),
        "wpp": wpp,
        "outw": out_w.astype(ml_dtypes.bfloat16),
    }

    in_maps = []
    for c in range(N_CORES):
        lo = c * NPC
        win, msl = per_core[c]
        slots = win * 128 + msl
        x0T = np.zeros((CH, SLOTS), dtype=ml_dtypes.bfloat16)
        x1T = np.zeros((CH, SLOTS), dtype=ml_dtypes.bfloat16)
        x0T[:, slots] = x0[lo:lo + NPC].T.astype(ml_dtypes.bfloat16)
        x1T[:, slots] = x1[lo:lo + NPC].T.astype(ml_dtypes.bfloat16)
        m = dict(shared)
        m["x0T"] = x0T
        m["x1T"] = x1T
        m["gidx"] = gidx[c]
        m["dcol"] = dstcol[c]
        in_maps.append(m)

    nc = _build_kernel(chunk_bound)
    _cached["in_maps"] = in_maps
    res = bass_utils.run_bass_kernel_spmd(
        nc, in_maps, core_ids=list(range(N_CORES)), trace=False)

    out = np.empty((N, C), dtype=np.float32)
    for c in range(N_CORES):
        lo = c * NPC
        win, msl = per_core[c]
        slots = win * 128 + msl
        out[lo:lo + NPC] = res.results[c]["o"][slots]
    out += out_b[None, :]
    return out
